# revision 13
# baseline (speedup 1.0000x reference)
"""Trainium2 Bass kernel for nn_MinimalLoss (YOLO-style detection loss).

Strategy (data-parallel over 8 NeuronCores, 4 batches each):
  * conf channel: 102400 4-byte strided gathers/core. This is a hard
    ~62us floor (16 SDMA engines x ~9.6ns/descriptor). The Sync engine
    is dedicated to issuing these 8 chunk DMAs back-to-back, starting
    as the very first kernel instruction.
  * everything else (per-target gather + losses, dedup, constants) is
    issued on gpsimd/DVE/ACT/PE and hides completely under the conf DMA.
  * all activation math uses only Exp/Ln (one ACT table, zero 1.28us
    table swaps) via softplus identities:
       ln sigma(x)      = -ln(1+e^-x)
       ln(1-sigma(x))   = -x - ln(1+e^-x)
       sigma(x)         = exp(-ln(1+e^-x))
       lnn - lnp        = -x      (conf correction term is just -logit)
       sum_cells ln(1-sigma) = -sum softplus = -sum ln(1+e^x)
  * final reduction on host: per-core outputs are acc [128, NCH]
    (per-partition softplus partial sums) and stats [100, 10]
    (per-target columns for both halves).
"""
import numpy as np

import concourse.bass as bass
import concourse.mybir as mybir
import concourse.tile as tile
from concourse.bass import IndirectOffsetOnAxis
from concourse.masks import make_identity

F32 = mybir.dt.float32
I32 = mybir.dt.int32
AF = mybir.ActivationFunctionType
ALU = mybir.AluOpType
AX = mybir.AxisListType

B, HWC, C, T = 32, 25600, 80, 50          # full problem
H = W = 160
NCORES = 8
BL = B // NCORES                          # 4 batches per core
ROWS = BL * HWC                           # 102400 prediction rows per core
NT = BL * T                               # 200 targets per core
HALF = NT // 2                            # 100 targets per half (2 batches)
MAGIC = float(np.float32(2 ** 23))
NCH = 8                                   # conf chunks
CHUNKS = [800 // NCH] * NCH
assert sum(CHUNKS) == 800 and len(CHUNKS) == NCH


def _floor(nc, sb, dst, src, n):
    """dst = floor(src) for 0 <= src < 2^22, exact (round-to-nearest fixup)."""
    r = sb.tile([n, 1], F32, tag="fl_r")
    adj = sb.tile([n, 1], F32, tag="fl_a")
    nc.vector.tensor_scalar_add(r[:], src, MAGIC)
    nc.vector.tensor_scalar_add(r[:], r[:], -MAGIC)
    nc.vector.tensor_tensor(out=adj[:], in0=r[:], in1=src, op=ALU.is_gt)
    nc.vector.tensor_tensor(out=dst, in0=r[:], in1=adj[:], op=ALU.subtract)


def _split_multi_waits(nc):
    """Walrus codegen accepts at most ONE sync wait per instruction; hoist
    extras onto standalone EventSemaphore (wait) ops on the same engine."""
    n = 0
    for func in nc.m.functions:
        for block in func.blocks:
            out = []
            for inst in block.instructions:
                si = inst.sync_info
                if si is not None and si.on_wait and len(si.on_wait) > 1:
                    waits = list(si.on_wait)
                    for w in waits[:-1]:
                        n += 1
                        nop = mybir.InstEventSemaphore(
                            name=f"{inst.name}_sw{n}", engine=inst.engine,
                            ins=[], outs=[])
                        nop.sync_info = mybir.SyncInfo(on_wait=[w], on_update=[])
                        out.append(nop)
                    inst.sync_info = mybir.SyncInfo(on_wait=[waits[-1]],
                                                    on_update=list(si.on_update))
                out.append(inst)
            if n:
                block.instructions[:] = out
    return n


def build_nc(split=True):
    nc = bass.Bass("TRN2", target_bir_lowering=False, debug=False)
    pred_d = nc.dram_tensor("predictions", [ROWS, 85], F32, kind="ExternalInput")
    tgt_d = nc.dram_tensor("targets", [NT, 5], F32, kind="ExternalInput")
    stats_d = nc.dram_tensor("stats", [HALF, 10], F32, kind="ExternalOutput")
    acc_d = nc.dram_tensor("acc", [128, NCH], F32, kind="ExternalOutput")

    pred_ap = pred_d.ap()

    with tile.TileContext(nc) as tc:
        with tc.tile_pool(name="persist", bufs=1) as pp, \
             tc.tile_pool(name="conf", bufs=NCH) as cp, \
             tc.tile_pool(name="sb", bufs=2) as sb, \
             tc.tile_pool(name="ps", bufs=1, space="PSUM") as ps:

            # ---- conf channel DMAs: the critical path. Sync engine does
            # nothing else; issues stream back-to-back from t~=0.
            conf = pred_ap[:, 4:5].rearrange("(p j) o -> p (j o)", p=128)  # [128, 800]
            conf_tl = []
            off = 0
            for k, cw in enumerate(CHUNKS):
                tl = cp.tile([128, cw], F32, tag=f"conf_in{k}")
                nc.sync.dma_start(out=tl[:], in_=conf[:, off:off + cw])
                conf_tl.append(tl)
                off += cw

            accT = pp.tile([128, NCH], F32)

            # ---- targets load early (gpsimd queue, independent of conf)
            # [100, 10]: half q in cols 5q..5q+4
            tt2 = pp.tile([HALF, 10], F32)
            nc.gpsimd.dma_start(out=tt2[:, 0:5], in_=tgt_d.ap()[0:HALF, :])
            nc.gpsimd.dma_start(out=tt2[:, 5:10], in_=tgt_d.ap()[HALF:NT, :])

            # ---- constants (gpsimd iota + DVE copies; matmul operands routed
            # through DVE so each PE op needs at most ONE sync wait)
            ident_g = pp.tile([128, 128], F32)
            make_identity(nc, ident_g[:])
            ident = pp.tile([128, 128], F32)
            nc.vector.tensor_copy(out=ident[:], in_=ident_g[:])
            iotac = pp.tile([128, C], I32)
            nc.gpsimd.iota(iotac[:], pattern=[[1, C]], base=0, channel_multiplier=0)
            iotaf = pp.tile([128, C], F32)
            nc.vector.tensor_copy(out=iotaf[:], in_=iotac[:])
            iotap = pp.tile([128, 1], I32)
            nc.gpsimd.iota(iotap[:], pattern=[[1, 1]], base=0, channel_multiplier=1)
            pf128 = pp.tile([128, 1], F32)
            nc.vector.tensor_copy(out=pf128[:], in_=iotap[:])
            iotar = pp.tile([128, 128], I32)
            nc.gpsimd.iota(iotar[:], pattern=[[1, 128]], base=0, channel_multiplier=0)
            iotarf = pp.tile([128, 128], F32)
            nc.vector.tensor_copy(out=iotarf[:], in_=iotar[:])
            tri = pp.tile([128, 128], F32)  # tri[p, f] = 1.0 iff f < p
            nc.vector.tensor_tensor(out=tri[:], in0=pf128[:].to_broadcast([128, 128]),
                                    in1=iotarf[:], op=ALU.is_gt)

            stats2 = pp.tile([HALF, 10], F32)

            # ---- per-target phase: two halves of 100 targets (2 whole batches
            # each), entirely hidden under the conf DMA stream.
            P = HALF
            for q in range(2):
                o = 5 * q
                xW = sb.tile([P, 1], F32, tag="xW")
                yH = sb.tile([P, 1], F32, tag="yH")
                nc.vector.tensor_scalar_mul(xW[:], tt2[:, o + 1:o + 2], float(W))
                nc.vector.tensor_scalar_mul(yH[:], tt2[:, o + 2:o + 3], float(H))
                gx = sb.tile([P, 1], F32, tag="gx")
                gy = sb.tile([P, 1], F32, tag="gy")
                _floor(nc, sb, gx[:], xW[:], P)
                _floor(nc, sb, gy[:], yH[:], P)

                # validity
                vf = sb.tile([P, 1], F32, tag="vf")
                tmp = sb.tile([P, 1], F32, tag="tmp")
                nc.vector.tensor_scalar(out=vf[:], in0=gx[:], scalar1=0.0, scalar2=None, op0=ALU.is_ge)
                nc.vector.tensor_scalar(out=tmp[:], in0=gx[:], scalar1=float(W), scalar2=None, op0=ALU.is_lt)
                nc.vector.tensor_tensor(out=vf[:], in0=vf[:], in1=tmp[:], op=ALU.mult)
                nc.vector.tensor_scalar(out=tmp[:], in0=gy[:], scalar1=0.0, scalar2=None, op0=ALU.is_ge)
                nc.vector.tensor_tensor(out=vf[:], in0=vf[:], in1=tmp[:], op=ALU.mult)
                nc.vector.tensor_scalar(out=tmp[:], in0=gy[:], scalar1=float(H), scalar2=None, op0=ALU.is_lt)
                nc.vector.tensor_tensor(out=vf[:], in0=vf[:], in1=tmp[:], op=ALU.mult)

                # cell + per-core row index
                gxi = sb.tile([P, 1], F32, tag="gxi")
                gyi = sb.tile([P, 1], F32, tag="gyi")
                nc.vector.tensor_scalar(out=gxi[:], in0=gx[:], scalar1=0.0, scalar2=float(W - 1),
                                        op0=ALU.max, op1=ALU.min)
                nc.vector.tensor_scalar(out=gyi[:], in0=gy[:], scalar1=0.0, scalar2=float(H - 1),
                                        op0=ALU.max, op1=ALU.min)
                cell = sb.tile([P, 1], F32, tag="cell")
                nc.vector.tensor_scalar_mul(cell[:], gyi[:], float(W))
                nc.vector.tensor_tensor(out=cell[:], in0=cell[:], in1=gxi[:], op=ALU.add)

                rowf = sb.tile([P, 1], F32, tag="rowf")
                # batch offset: (2q + (t>=50)) * HWC
                nc.vector.tensor_scalar(out=rowf[:], in0=pf128[:P, :], scalar1=float(T), scalar2=None,
                                        op0=ALU.is_ge)
                nc.vector.tensor_scalar(out=rowf[:], in0=rowf[:], scalar1=float(HWC),
                                        scalar2=float(2 * q * HWC), op0=ALU.mult, op1=ALU.add)
                nc.vector.tensor_tensor(out=rowf[:], in0=rowf[:], in1=cell[:], op=ALU.add)
                idx = sb.tile([P, 1], I32, tag="idx")
                nc.vector.tensor_copy(out=idx[:], in_=rowf[:])

                # dedup key: valid -> rowf ; invalid -> unique negative
                negk = sb.tile([P, 1], F32, tag="negk")
                nc.vector.tensor_scalar(out=negk[:], in0=pf128[:P, :], scalar1=-1.0,
                                        scalar2=-(1.0 + 100.0 * q), op0=ALU.mult, op1=ALU.add)
                key = sb.tile([P, 1], F32, tag="key")
                nc.vector.tensor_tensor(out=key[:], in0=rowf[:], in1=negk[:], op=ALU.subtract)
                nc.vector.tensor_tensor(out=key[:], in0=key[:], in1=vf[:], op=ALU.mult)
                nc.vector.tensor_tensor(out=key[:], in0=key[:], in1=negk[:], op=ALU.add)

                # gather prediction rows
                rows = sb.tile([P, 85], F32, tag="rows")
                nc.gpsimd.indirect_dma_start(
                    out=rows[:], out_offset=None, in_=pred_ap[:, :],
                    in_offset=IndirectOffsetOnAxis(ap=idx[:, :1], axis=0))

                # cls: bce_sum = sum_c(x_c + s_c) - x_{c*},  s_c = ln(1+e^-x_c)
                ecls = sb.tile([P, C], F32, tag="ecls")
                nc.scalar.activation(out=ecls[:], in_=rows[:, 5:85], func=AF.Exp, scale=-1.0)
                scls = sb.tile([P, C], F32, tag="scls")
                ssum = sb.tile([P, 1], F32, tag="ssum")
                nc.scalar.activation(out=scls[:], in_=ecls[:], func=AF.Ln, bias=1.0, accum_out=ssum[:])
                xsum = sb.tile([P, 1], F32, tag="xsum")
                nc.vector.reduce_sum(out=xsum[:], in_=rows[:, 5:85], axis=AX.X)
                oh = sb.tile([P, C], F32, tag="oh")
                nc.vector.tensor_tensor(out=oh[:], in0=iotaf[:P, :],
                                        in1=tt2[:, o:o + 1].to_broadcast([P, C]), op=ALU.is_equal)
                ohx = sb.tile([P, C], F32, tag="ohx")
                nc.vector.tensor_tensor(out=ohx[:], in0=oh[:], in1=rows[:, 5:85], op=ALU.mult)
                xstar = sb.tile([P, 1], F32, tag="xstar")
                nc.vector.reduce_sum(out=xstar[:], in_=ohx[:], axis=AX.X)
                pcls = sb.tile([P, 1], F32, tag="pcls")
                nc.vector.tensor_tensor(out=pcls[:], in0=ssum[:], in1=xsum[:], op=ALU.add)
                nc.vector.tensor_tensor(out=pcls[:], in0=pcls[:], in1=xstar[:], op=ALU.subtract)
                nc.vector.tensor_scalar_mul(pcls[:], pcls[:], 1.0 / C)

                # xy: sigma(x) = exp(-ln(1+e^-x))
                exy = sb.tile([P, 2], F32, tag="exy")
                nc.scalar.activation(out=exy[:], in_=rows[:, 0:2], func=AF.Exp, scale=-1.0)
                sxy = sb.tile([P, 2], F32, tag="sxy")
                nc.scalar.activation(out=sxy[:], in_=exy[:], func=AF.Ln, bias=1.0)
                sigxy = sb.tile([P, 2], F32, tag="sigxy")
                nc.scalar.activation(out=sigxy[:], in_=sxy[:], func=AF.Exp, scale=-1.0)
                txy = sb.tile([P, 2], F32, tag="txy")
                nc.vector.tensor_tensor(out=txy[:, 0:1], in0=xW[:], in1=gx[:], op=ALU.subtract)
                nc.vector.tensor_tensor(out=txy[:, 1:2], in0=yH[:], in1=gy[:], op=ALU.subtract)
                dxy = sb.tile([P, 2], F32, tag="dxy")
                nc.vector.tensor_tensor(out=dxy[:], in0=sigxy[:], in1=txy[:], op=ALU.subtract)
                nc.vector.tensor_tensor(out=dxy[:], in0=dxy[:], in1=dxy[:], op=ALU.mult)
                pxy = sb.tile([P, 1], F32, tag="pxy")
                nc.vector.reduce_sum(out=pxy[:], in_=dxy[:], axis=AX.X)
                nc.vector.tensor_scalar_mul(pxy[:], pxy[:], 0.5)

                # wh
                pwh_t = sb.tile([P, 2], F32, tag="pwh")
                nc.scalar.activation(out=pwh_t[:], in_=rows[:, 2:4], func=AF.Exp)
                twh = sb.tile([P, 2], F32, tag="twh")
                nc.vector.tensor_scalar_mul(twh[:, 0:1], tt2[:, o + 3:o + 4], float(W))
                nc.vector.tensor_scalar_mul(twh[:, 1:2], tt2[:, o + 4:o + 5], float(H))
                dwh = sb.tile([P, 2], F32, tag="dwh")
                nc.vector.tensor_tensor(out=dwh[:], in0=pwh_t[:], in1=twh[:], op=ALU.subtract)
                nc.vector.tensor_tensor(out=dwh[:], in0=dwh[:], in1=dwh[:], op=ALU.mult)
                pwh = sb.tile([P, 1], F32, tag="pwh1")
                nc.vector.reduce_sum(out=pwh[:], in_=dwh[:], axis=AX.X)
                nc.vector.tensor_scalar_mul(pwh[:], pwh[:], 0.5)

                # dedup: first-occurrence weight w (for obj_mask scatter-max)
                keyT_ps = ps.tile([P, P], F32, space="PSUM", tag="keyT_ps")
                nc.tensor.transpose(out=keyT_ps[:], in_=key[:].to_broadcast([P, P]),
                                    identity=ident[:P, :P])
                keyT = sb.tile([P, P], F32, tag="keyT")
                nc.vector.tensor_copy(out=keyT[:], in_=keyT_ps[:])
                eq = sb.tile([P, P], F32, tag="eq")
                nc.vector.tensor_tensor(out=eq[:], in0=key[:].to_broadcast([P, P]),
                                        in1=keyT[:], op=ALU.is_equal)
                nc.vector.tensor_tensor(out=eq[:], in0=eq[:], in1=tri[:P, :P], op=ALU.mult)
                dup = sb.tile([P, 1], F32, tag="dup")
                nc.vector.reduce_max(out=dup[:], in_=eq[:], axis=AX.X)
                wfo = sb.tile([P, 1], F32, tag="wfo")
                nc.vector.tensor_scalar(out=wfo[:], in0=dup[:], scalar1=-1.0, scalar2=1.0,
                                        op0=ALU.mult, op1=ALU.add)
                nc.vector.tensor_tensor(out=wfo[:], in0=wfo[:], in1=vf[:], op=ALU.mult)

                # stats columns: vf*pxy, vf*pwh, vf*pcls, vf, wfo*x4 (host negates)
                nc.vector.tensor_tensor(out=stats2[:, o + 0:o + 1], in0=pxy[:], in1=vf[:], op=ALU.mult)
                nc.vector.tensor_tensor(out=stats2[:, o + 1:o + 2], in0=pwh[:], in1=vf[:], op=ALU.mult)
                nc.vector.tensor_tensor(out=stats2[:, o + 2:o + 3], in0=pcls[:], in1=vf[:], op=ALU.mult)
                nc.vector.tensor_copy(out=stats2[:, o + 3:o + 4], in_=vf[:])
                nc.vector.tensor_tensor(out=stats2[:, o + 4:o + 5], in0=rows[:, 4:5], in1=wfo[:], op=ALU.mult)

            # stats out on gpsimd: fully hidden under the conf stream
            nc.gpsimd.dma_start(out=stats_d.ap()[:, :], in_=stats2[:])

            # ---- conf compute: softplus(x) = ln(1+e^x) summed per partition.
            # ln(1+t) via the activation's pre-function bias: Ln(t*1 + 1) —
            # keeps the whole chain on the Scalar engine (no DVE hop).
            for k, cw in enumerate(CHUNKS):
                ex = cp.tile([128, cw], F32, tag=f"conf_ex{k}")
                nc.scalar.activation(out=ex[:], in_=conf_tl[k][:], func=AF.Exp)
                lnt = cp.tile([128, cw], F32, tag=f"conf_ln{k}")
                nc.scalar.activation(out=lnt[:], in_=ex[:], func=AF.Ln, bias=1.0,
                                     accum_out=accT[:, k:k + 1])

            # acc out on scalar: follows ln7 in program order on the same
            # engine (no cross-engine waits) and uses the empty ACT HWDGE ring
            nc.scalar.dma_start(out=acc_d.ap()[:, :], in_=accT[:])
    if split:
        _split_multi_waits(nc)
    return nc


_NC_CACHE = None


def _get_nc():
    global _NC_CACHE
    if _NC_CACHE is None:
        _NC_CACHE = build_nc()
    return _NC_CACHE


def make_in_maps(predictions, targets):
    preds = np.ascontiguousarray(np.asarray(predictions, dtype=np.float32)).reshape(NCORES, ROWS, 85)
    tgts = np.ascontiguousarray(np.asarray(targets, dtype=np.float32)).reshape(NCORES, NT, 5)
    return [{"predictions": preds[c], "targets": tgts[c]} for c in range(NCORES)]


def combine_partials(results):
    """results: list of 8 dicts with 'stats' [100,10] and 'acc' [128,NCH]
    -> (total, loss_xy, loss_wh, loss_conf, loss_cls)"""
    st = np.sum([np.asarray(r["stats"], dtype=np.float64) for r in results], axis=(0, 1))
    sp_total = float(np.sum([np.asarray(r["acc"], dtype=np.float64) for r in results]))
    xy = st[0] + st[5]
    wh = st[1] + st[6]
    cls_ = st[2] + st[7]
    nt = st[3] + st[8]
    x4 = st[4] + st[9]
    denom = np.float32(max(float(nt), 1.0))
    loss_xy = np.float32(np.float32(xy) / denom)
    loss_wh = np.float32(np.float32(wh) / denom)
    loss_cls = np.float32(np.float32(cls_) / denom)
    loss_conf = np.float32((np.float32(sp_total) - np.float32(x4)) / np.float32(B * HWC))
    total = np.float32(5.0 * loss_xy + 5.0 * loss_wh + loss_conf + loss_cls)
    return total, loss_xy, loss_wh, loss_conf, loss_cls


def kernel(predictions, targets, H=None, W=None):
    from concourse.bass_utils import run_bass_kernel_spmd

    nc = _get_nc()
    in_maps = make_in_maps(predictions, targets)
    res = run_bass_kernel_spmd(nc, in_maps, core_ids=list(range(NCORES)))
    return combine_partials([res.results[c] for c in range(NCORES)])
